# revision 2
# baseline (speedup 1.0000x reference)
"""Trainium2 Bass kernel for nn_MultiHeadAttention_43971875177057 (v2).

MHA with residual: B=2, S=4096, d_model=512, n_heads=8, dk=64.
Sharding: 8 cores = (batch b) x (head-pair hp); each core computes 2 heads
over the full sequence and owns output rows [hp*1024, (hp+1)*1024) of
batch b (the reference's head-interleaving reshape makes that row range
depend only on those 2 heads).

Key optimizations over v1 (PE-bound at ~443us with a mid-p-state PE):
 * All matmuls run in fp8(e4m3) DoubleRow perf mode: the PE processes two
   moving values per cycle (0.5 cycles/row), halving stream time.  The
   hardware requires contiguous free APs on the operands, so every tensor
   is laid out with its DoubleRow pair dim innermost-but-one (host-side
   for x/W*, via partition-shift SBUF DMAs for Q2/K2, via strided engine
   writes for at/ctx).
 * The softmax exp (33.5M elems/core; only the Activation engine has an
   exp table) is split: Act computes real exp->fp8 for ~56% of score
   tiles; DVE computes the fp8 BITS for the rest with a Schraudolph
   trick: bits = (z max 0) * mask -> int8, where z = 8*log2e*score + K0
   arrives pre-scaled from the QK matmul (log2e folded into Q, K0 via a
   bias row in the 33-partition extended contraction).  That single fused
   scalar_tensor_tensor also applies the mask.
 * Act tiles get masked separately: bitwise-AND on uint32 views (DVE;
   HW supports bitwise only on DVE @32bit) or fp8 multiply on GpSimd
   (which cannot touch PSUM, so this is its only useful contribution).
   The host encodes each kt block of the mask in the encoding its
   masking engine needs (0xFF bytes for AND, fp8 1.0 for multiplies), so
   the mask ships once, as bytes: 16 MiB/core, half of v1.
 * QKV conversions run on Act (Identity with per-partition bias AP for
   Q/K; Copy for V, whose column bias is folded into an extra rank-1
   matmul row), keeping DVE free for its exp share.
 * Softmax denominators fall out of the ctx matmul via a ones-column in
   V; the reciprocal is partition-broadcast through a DRAM bounce.
"""

import os
import sys
import types

import numpy as np
import ml_dtypes

B, S, D, H, DK = 2, 4096, 512, 8, 64
QC = 1024           # queries per chunk
NQC = S // QC       # 4 chunks
NKT = S // 128      # 32 key tiles
RT = S // 8         # 512 output rows per head
BF8 = ml_dtypes.float8_e4m3fn
LOG2E = 1.4426950408889634
WS = 16.0           # host weight prescale (undone in PSUM->fp8 conversions)
K0 = 38.25          # Schraudolph bias: 5.5*7.0 + (-0.5)*0.5 (fp8-exact rows)
SHIFT = 1.5         # at = exp(s - SHIFT); cancels in the softmax ratio
ACT_SCALE = 1.0 / (8.0 * LOG2E)
ACT_BIAS = -SHIFT - K0 * ACT_SCALE


def _make_plan():
    """Per-kt (exp_engine, mask_mode): 18 'A' (7 'V' DVE-AND + 11 'P'
    Pool-mult) and 14 'D' (fused mask) per 32, interleaved for smooth
    pipelining. Shared by device build and host mask encoding."""
    w = {"A": 18, "D": 14}
    used = {"A": 0, "D": 0}
    exp = []
    for i in range(NKT):
        e = max("AD", key=lambda k: w[k] * (i + 1) / NKT - used[k])
        used[e] += 1
        exp.append(e)
    wm = {"V": 7, "P": 11}
    um = {"V": 0, "P": 0}
    plan = []
    na = 0
    for e in exp:
        if e == "D":
            plan.append(("D", "F"))
            continue
        na += 1
        m = max("VP", key=lambda k: wm[k] * na / 18.0 - um[k])
        um[m] += 1
        plan.append(("A", m))
    return plan


PLAN = _make_plan()


def _build_kernel(n_cores=8):
    import concourse.bacc as bacc
    import concourse.mybir as mybir
    import concourse.tile as tile
    import concourse.bass as bass

    f32 = mybir.dt.float32
    fp8 = mybir.dt.float8e4
    i8 = mybir.dt.int8
    u8 = mybir.dt.uint8
    u32 = mybir.dt.uint32
    DR = mybir.MatmulPerfMode.DoubleRow
    Exp = mybir.ActivationFunctionType.Exp
    Ident = mybir.ActivationFunctionType.Identity
    Cpy = mybir.ActivationFunctionType.Copy
    Max = mybir.AluOpType.max
    Mul = mybir.AluOpType.mult
    Add = mybir.AluOpType.add
    And = mybir.AluOpType.bitwise_and

    nc = bacc.Bacc("TRN2", target_bir_lowering=False, debug=False,
                   num_devices=n_cores)

    # -- DRAM inputs (per-core layouts prepared by _shard_inputs) ---------
    # x2: [p, g, i, s] with x^T[c, s]: c = g*256+i*128+p (DoubleRow pairs)
    x2d = nc.dram_tensor("x2", [128, 2 * 2 * S], fp8,
                         kind="ExternalInput").ap()
    # x2q: same layout but with the query columns PERMUTED within each
    # quarter (col j*128+t <- query 8t+j) so that attention/ctx columns come
    # out j-major and the out-projection lhsT pair slices are contiguous
    # without any strided elementwise ops.  Only the Q projection uses it.
    x2qd = nc.dram_tensor("x2q", [128, 2 * 2 * S], fp8,
                          kind="ExternalInput").ap()
    mskd = nc.dram_tensor("msk", [128, NQC * NKT * 1024], u8,
                          kind="ExternalInput").ap()
    wq2d = nc.dram_tensor("wq2", [128, 2 * 2 * 128], fp8, kind="ExternalInput").ap()
    wk2d = nc.dram_tensor("wk2", [128, 2 * 2 * 128], fp8, kind="ExternalInput").ap()
    wv2d = nc.dram_tensor("wv2", [128, 2 * 2 * 144], fp8, kind="ExternalInput").ap()
    wo2d = nc.dram_tensor("wo2", [64, 4 * 2 * 512], fp8, kind="ExternalInput").ap()
    bq2d = nc.dram_tensor("bq2", [128, 1], f32, kind="ExternalInput").ap()
    bkd = nc.dram_tensor("bk", [128, 1], f32, kind="ExternalInput").ap()
    bvrd = nc.dram_tensor("bvrow", [1, 2 * 144], fp8, kind="ExternalInput").ap()
    q2rd = nc.dram_tensor("q2row", [1, 2 * QC], fp8,
                          kind="ExternalInput").ap()
    k2rd = nc.dram_tensor("k2row", [1, 2 * S], fp8,
                          kind="ExternalInput").ap()
    xrd = nc.dram_tensor("xresb", [2 * RT, D], f32, kind="ExternalInput").ap()
    outd = nc.dram_tensor("out", [2 * RT, D], f32, kind="ExternalOutput").ap()
    rc_dram = nc.dram_tensor("rc_scratch", [2 * NQC, QC], f32).ap()
    sum_dram = nc.dram_tensor("sum_scratch", [2 * NQC, QC], f32).ap()

    def pbcast(ap, p):
        return bass.AP(tensor=ap.tensor, offset=ap.offset,
                       ap=[[0, p]] + list(ap.ap[1:]))

    msk4 = mskd.rearrange("p (c k q) -> p c k q", c=NQC, k=NKT)

    with tile.TileContext(nc) as tc:
        with (
            tc.tile_pool(name="const", bufs=1) as const,
            tc.tile_pool(name="mask", bufs=2) as maskp,
            tc.tile_pool(name="attn", bufs=2) as attnp,
            tc.tile_pool(name="qk2", bufs=2) as qk2p,
            tc.tile_pool(name="stage", bufs=2) as stagep,
            tc.tile_pool(name="small", bufs=2) as small,
            tc.tile_pool(name="outp", bufs=2) as outp,
            tc.tile_pool(name="psum", bufs=1, space="PSUM") as psum,
        ):
            # ---- constant loads ----------------------------------------
            x2 = const.tile([128, 2, 2, S], fp8)
            nc.sync.dma_start(out=x2, in_=x2d.rearrange(
                "p (g i s) -> p g i s", g=2, i=2))
            x2q = const.tile([128, 2, 2, S], fp8)
            nc.sync.dma_start(out=x2q, in_=x2qd.rearrange(
                "p (g i s) -> p g i s", g=2, i=2))
            wq2 = const.tile([128, 2, 2, 128], fp8)
            nc.sync.dma_start(out=wq2, in_=wq2d.rearrange(
                "p (g i m) -> p g i m", g=2, i=2))
            wk2 = const.tile([128, 2, 2, 128], fp8)
            nc.sync.dma_start(out=wk2, in_=wk2d.rearrange(
                "p (g i m) -> p g i m", g=2, i=2))
            wv2 = const.tile([128, 2, 2, 144], fp8)
            nc.sync.dma_start(out=wv2, in_=wv2d.rearrange(
                "p (g i m) -> p g i m", g=2, i=2))
            wo2 = const.tile([64, 4, 2, 512], fp8)
            nc.sync.dma_start(out=wo2, in_=wo2d.rearrange(
                "p (j i e) -> p j i e", j=4, i=2))
            bq2 = const.tile([128, 1], f32)
            nc.sync.dma_start(out=bq2, in_=bq2d)
            bk = const.tile([128, 1], f32)
            nc.sync.dma_start(out=bk, in_=bkd)
            bvrow = const.tile([1, 2, 144], fp8)
            nc.sync.dma_start(out=bvrow,
                              in_=bvrd.rearrange("o (i m) -> o i m", i=2))
            ones1 = const.tile([1, 2, 128], fp8)
            nc.gpsimd.memset(ones1[:, 0, :], 1.0)
            nc.gpsimd.memset(ones1[:, 1, :], 0.0)
            actb = const.tile([128, 1], f32)
            nc.gpsimd.memset(actb, ACT_BIAS)

            # K2 per head: [33, kt, i, 128] fp8 (kt-major so scores lhsT
            # slices are contiguous); row 32 = Schraudolph K0 bias row
            K2 = [const.tile([33, 2, S], fp8, tag=f"k2_{h}",
                             name=f"k2_{h}") for h in (0, 1)]
            for h in (0, 1):
                nc.sync.dma_start(out=K2[h][32:33, :, :],
                                  in_=k2rd.rearrange("o (i s) -> o i s", i=2))
            # V: [128, ktp, h, i, 80] fp8, cols 0..64 used (64 = ones);
            # pair stride 80 satisfies the DoubleRow LDWEIGHTS step%16==0
            vsb = const.tile([128, NKT // 2, 2, 2, 80], fp8, tag="v",
                             name="v")

            # ---- per-quarter producers ---------------------------------
            def produce_quarter(qc):
                q0 = qc * QC
                psq = psum.tile([128, QC], f32, tag="sps0", name="psq")
                psk = psum.tile([128, QC], f32, tag="sps1", name="psk")
                for hf in (0, 1):
                    for g in (0, 1):
                        nc.tensor.matmul(
                            psq[:, hf * 512:(hf + 1) * 512], lhsT=wq2[:, g],
                            rhs=x2q[:, g, :,
                                    q0 + hf * 512:q0 + (hf + 1) * 512],
                            start=(g == 0), stop=(g == 1), perf_mode=DR)
                    for g in (0, 1):
                        nc.tensor.matmul(
                            psk[:, hf * 512:(hf + 1) * 512], lhsT=wk2[:, g],
                            rhs=x2[:, g, :,
                                   q0 + hf * 512:q0 + (hf + 1) * 512],
                            start=(g == 0), stop=(g == 1), perf_mode=DR)
                qf8 = stagep.tile([128, QC], fp8, tag="qf8")
                nc.scalar.activation(qf8, psq, Ident, bias=bq2,
                                     scale=LOG2E / WS)
                kf8 = stagep.tile([128, QC], fp8, tag="kf8")
                nc.scalar.activation(kf8, psk, Ident, bias=bk, scale=1.0 / WS)
                # shuffle into DoubleRow pair layouts (partition-shift DMAs)
                q0 = qc * QC
                q2t = []
                for h in (0, 1):
                    q2 = qk2p.tile([33, 2, QC], fp8, tag=f"q2_{h}")
                    for i in (0, 1):
                        nc.sync.dma_start(
                            out=q2[0:32, i, :],
                            in_=qf8[h * 64 + 32 * i:h * 64 + 32 * i + 32, :])
                        nc.sync.dma_start(
                            out=K2[h][0:32, i, q0:q0 + QC],
                            in_=kf8[h * 64 + 32 * i:h * 64 + 32 * i + 32, :])
                    nc.sync.dma_start(
                        out=q2[32:33, :, :],
                        in_=q2rd.rearrange("o (i s) -> o i s", i=2))
                    q2t.append(q2)
                # V (bias added via rank-1 matmul row, conversion on Act)
                for kt in range(qc * 8, qc * 8 + 8):
                    psv = psum.tile([128, 130], f32, tag="sps2", name="psv")
                    for g in (0, 1):
                        nc.tensor.matmul(
                            psv,
                            lhsT=x2[:, g, :, kt * 128:(kt + 1) * 128],
                            rhs=wv2[:, g, :, 0:130],
                            start=(g == 0), stop=False, perf_mode=DR)
                    nc.tensor.matmul(psv, lhsT=ones1, rhs=bvrow[:, :, 0:130],
                                     start=False, stop=True, perf_mode=DR)
                    nc.scalar.activation(
                        vsb[:, kt // 2, :, kt % 2, 0:65],
                        psv.rearrange("p (h c) -> p h c", h=2),
                        Cpy, bias=0.0, scale=1.0 / WS)
                return q2t

            # ---- attention pass for one (qc, h) ------------------------
            def attention(qc, h, q2, mslabs):
                ctx = psum.tile([65, QC], f32, tag="ctx", name=f"ctx{h}")
                at2s = {}

                def emit_ctx(ktp):
                    a = at2s.pop(ktp)
                    for hf in (0, 1):
                        nc.tensor.matmul(
                            ctx[:, hf * 512:(hf + 1) * 512],
                            lhsT=vsb[:, ktp, h, :, 0:65],
                            rhs=a[:, :, hf * 512:(hf + 1) * 512],
                            start=(ktp == 0), stop=(ktp == NKT // 2 - 1),
                            perf_mode=DR)

                for kt in range(NKT):
                    sps = psum.tile([128, QC], f32, tag=f"sps{kt % 3}")
                    for hf in (0, 1):
                        nc.tensor.matmul(
                            sps[:, hf * 512:(hf + 1) * 512],
                            lhsT=K2[h][:, :, kt * 128:(kt + 1) * 128],
                            rhs=q2[:, :, hf * 512:(hf + 1) * 512],
                            start=True, stop=True, perf_mode=DR)
                    if kt % 2 == 0:
                        at2s[kt // 2] = attnp.tile(
                            [128, 2, QC], fp8, tag=f"at{(kt // 2) % 4}",
                            name=f"at2_{kt // 2}")
                    sec = at2s[kt // 2][:, kt % 2, :]
                    msec = mslabs[kt // 16][:, kt % 16, :]
                    e, m = PLAN[kt]
                    if e == "A":
                        nc.scalar.activation(sec, sps, Exp, bias=actb,
                                             scale=ACT_SCALE)
                        if m == "V":
                            nc.vector.tensor_tensor(
                                sec.bitcast(u32), sec.bitcast(u32),
                                msec.bitcast(u32), And)
                        else:
                            nc.gpsimd.tensor_tensor(
                                sec, sec, msec.bitcast(fp8), Mul)
                    else:
                        nc.vector.scalar_tensor_tensor(
                            sec.bitcast(i8), sps, 0.0, msec.bitcast(fp8),
                            Max, Mul)
                    # lag the ctx matmuls ~4 kt behind the scores so the
                    # in-order PE queue never stalls on elementwise results
                    if kt % 2 == 1 and kt >= 7:
                        emit_ctx((kt - 7) // 2)
                for ktp in sorted(at2s):
                    emit_ctx(ktp)

                # softmax normalize: 1/sums broadcast via DRAM bounce
                srow = small.tile([1, QC], f32, tag="srow")
                nc.scalar.copy(srow, ctx[64:65, :])
                sraw = sum_dram[qc * 2 + h:qc * 2 + h + 1, :]
                nc.sync.dma_start(out=sraw, in_=srow)
                sums = small.tile([128, QC // 128], f32, tag="sums")
                nc.sync.dma_start(
                    out=sums, in_=sraw.rearrange("o (p f) -> (o p) f", p=128))
                rc = small.tile([128, QC // 128], f32, tag="rc")
                nc.vector.reciprocal(rc, sums)
                row = rc_dram[qc * 2 + h:qc * 2 + h + 1, :]
                nc.sync.dma_start(
                    out=row.rearrange("o (p f) -> (o p) f", p=128), in_=rc)
                rcr = small.tile([64, QC], f32, tag="rcr")
                nc.sync.dma_start(out=rcr, in_=pbcast(row, 64))
                # ctx columns are already j-major (q-permuted host layout),
                # so the normalize is a plain contiguous multiply and the
                # out-proj pair slices are contiguous with stride 128
                ctxseg = outp.tile([64, QC], fp8, tag="ctxseg")
                nc.vector.tensor_mul(ctxseg, ctx[0:64, :], rcr)

                # output projection (fp8 DoubleRow over j-pairs) + residual
                ctx3 = ctxseg.rearrange("p (j t) -> p j t", j=8)
                ops = psum.tile([128, D], f32, tag="ctx", name=f"ops{h}")
                for jp in range(4):
                    nc.tensor.matmul(ops, lhsT=ctx3[:, 2 * jp:2 * jp + 2, :],
                                     rhs=wo2[:, jp],
                                     start=(jp == 0), stop=(jp == 3),
                                     perf_mode=DR)
                r0 = h * RT + qc * (QC // 8)
                xr = outp.tile([128, D], f32, tag="xr")
                nc.sync.dma_start(out=xr, in_=xrd[r0:r0 + 128, :])
                osb = outp.tile([128, D], f32, tag="osb")
                nc.vector.scalar_tensor_tensor(osb, ops, 1.0 / WS, xr,
                                               Mul, Add)
                nc.sync.dma_start(out=outd[r0:r0 + 128, :], in_=osb)

            # ---- main schedule -----------------------------------------
            # All quarters must be produced before any attention pass: the
            # kt loop reads K2/V over the full key range.  The scheduler
            # still overlaps production with attention via tile deps.
            q2all, msall = [], []
            for qc in range(NQC):
                ms = []
                for half in (0, 1):
                    mt = maskp.tile([128, 16, QC], u8, tag=f"ms{half}")
                    nc.sync.dma_start(
                        out=mt, in_=msk4[:, qc, half * 16:(half + 1) * 16, :])
                    ms.append(mt)
                msall.append(ms)
                q2all.append(produce_quarter(qc))
            for qc in range(NQC):
                for h in (0, 1):
                    attention(qc, h, q2all[qc][h], msall[qc])

    nc.compile()
    return nc


def _shard_inputs(x, mask, Wq, bq, Wk, bk, Wv, bv, Wo, bo):
    """Host-side marshaling: slice/transpose/cast per core. core = b*4+hp."""
    # mask slab, shared by all cores: [128, qc, kt, 1024] bytes, encoded
    # per kt block: 0xFF for AND-masked tiles, fp8 1.0 (0x38) for the rest
    # query-column permutation within each quarter: col j*128+t <- q 8t+j
    qperm = (np.arange(NQC)[:, None, None] * QC
             + 8 * np.arange(128)[None, None, :]
             + np.arange(8)[None, :, None]).reshape(-1)
    keepT = np.ascontiguousarray((1 - mask[0, 0]).T).astype(np.uint8)
    keepT = keepT[:, qperm]
    slab = keepT.reshape(NKT, 128, NQC, QC).transpose(1, 2, 0, 3)
    enc = np.array([0xFF if pm == "V" else 0x38 for (_, pm) in PLAN], np.uint8)
    slab = np.ascontiguousarray(slab * enc[None, None, :, None]).reshape(128, -1)

    def pair_w(wT, n):
        # [512, n] -> [128, 2, 2, n] with [p, g, i, m] = wT[g*256+i*128+p, m]
        return np.ascontiguousarray(
            wT.reshape(2, 2, 128, n).transpose(2, 0, 1, 3)
        ).astype(BF8).reshape(128, 2 * 2 * n)

    woT = (Wo.T * WS).astype(np.float32)
    wo2 = np.ascontiguousarray(
        woT.reshape(4, 2, 64, D).transpose(2, 0, 1, 3)).astype(BF8).reshape(64, -1)
    q2row = np.zeros((1, 2, QC), np.float32)
    q2row[:, 0, :] = 7.0
    q2row[:, 1, :] = 0.5
    k2row = np.zeros((1, 2, S), np.float32)
    k2row[:, 0, :] = 5.5
    k2row[:, 1, :] = -0.5

    in_maps = []
    for core in range(8):
        b, hp = divmod(core, 4)
        c0 = hp * 128
        xT = np.ascontiguousarray(x[b].T)  # [512, S]
        # [p, g, i, s]
        x2 = np.ascontiguousarray(
            xT.reshape(2, 2, 128, S).transpose(2, 0, 1, 3)
        ).astype(BF8).reshape(128, -1)
        xTq = xT[:, qperm]
        x2q = np.ascontiguousarray(
            xTq.reshape(2, 2, 128, S).transpose(2, 0, 1, 3)
        ).astype(BF8).reshape(128, -1)
        wvT_ext = np.zeros((D, 144), np.float32)
        wvT_ext[:, 0:64] = Wv[c0:c0 + 64, :].T * WS
        wvT_ext[:, 65:129] = Wv[c0 + 64:c0 + 128, :].T * WS
        bvrow = np.zeros((1, 2, 144), np.float32)
        bvrow[0, 0, 0:64] = bv[c0:c0 + 64] * WS
        bvrow[0, 0, 64] = WS
        bvrow[0, 0, 65:129] = bv[c0 + 64:c0 + 128] * WS
        bvrow[0, 0, 129] = WS
        in_maps.append({
            "x2": x2,
            "x2q": x2q,
            "msk": slab,
            "wq2": pair_w(np.ascontiguousarray(Wq[c0:c0 + 128, :].T * WS), 128),
            "wk2": pair_w(np.ascontiguousarray(Wk[c0:c0 + 128, :].T * WS), 128),
            "wv2": pair_w(wvT_ext, 144),
            "wo2": wo2,
            "bq2": (bq[c0:c0 + 128] * LOG2E).reshape(128, 1).astype(np.float32),
            "bk": bk[c0:c0 + 128].reshape(128, 1).astype(np.float32),
            "bvrow": bvrow.astype(BF8).reshape(1, -1),
            "q2row": q2row.astype(BF8).reshape(1, -1),
            "k2row": k2row.astype(BF8).reshape(1, -1),
            "xresb": np.ascontiguousarray(
                x[b, hp * 2 * RT:(hp + 1) * 2 * RT, :] + bo[None, :]
            ).astype(np.float32),
        })
    return in_maps


_RESULT_CACHE = {}


def _ensure_env():
    """Make concourse importable and register the NTFF profile hook."""
    for p in ("/root/.axon_site/_ro/trn_rl_repo", "/opt/trn_rl_repo"):
        if os.path.isdir(p) and p not in sys.path:
            sys.path.append(p)
    try:
        import antenv  # noqa: F401
        import antenv.axon_hooks  # noqa: F401
    except ImportError:
        try:
            import antenv
            mod = types.ModuleType("antenv.axon_hooks")
            _hook = [None]
            mod.set_axon_ntff_profile_hook = lambda h: _hook.__setitem__(0, h)
            mod.get_axon_ntff_profile_hook = lambda: _hook[0]
            sys.modules["antenv.axon_hooks"] = mod
            antenv.axon_hooks = mod
            from trn_agent_boot.trn_boot import _ntff_profile_via_ctypes
            so = "/opt/axon/libaxon_pjrt.so"
            if os.path.exists(so):
                mod.set_axon_ntff_profile_hook(_ntff_profile_via_ctypes(so))
        except Exception:
            pass


def kernel(x, mask, Wq, bq, Wk, bk, Wv, bv, Wo, bo, trace=False):
    _ensure_env()
    from concourse.bass_utils import run_bass_kernel_spmd

    x = np.asarray(x, np.float32)
    mask = np.asarray(mask)
    args = [np.asarray(a, np.float32) for a in (Wq, bq, Wk, bk, Wv, bv, Wo, bo)]
    nc = _RESULT_CACHE.get("nc")
    if nc is None:
        nc = _build_kernel()
        _RESULT_CACHE["nc"] = nc
    in_maps = _shard_inputs(x, mask, *args)
    res = run_bass_kernel_spmd(nc, in_maps, core_ids=list(range(8)),
                               trace=trace)
    _RESULT_CACHE["last_run"] = res
    out = np.empty((B, S, D), np.float32)
    for core in range(8):
        b, hp = divmod(core, 4)
        out[b, hp * 2 * RT:(hp + 1) * 2 * RT, :] = res.results[core]["out"]
    return out


if __name__ == "__main__":
    _ensure_env()
    nc = _build_kernel()
    print("kernel built + compiled OK")
